# revision 1
# baseline (speedup 1.0000x reference)
"""Block-causal sparse attention (QKNorm + RoPE) for Trainium2, 8 NeuronCores.

Sharding: batch*head parallel. 2 batches x 16 heads = 32 (b,h) pairs; core c
handles batch c//4, heads 4*(c%4) .. 4*(c%4)+4. The out-projection is computed
as per-core partials over the local head channels and summed with ReduceScatter
over each batch's 4-core group (chunked by query-half so the collective
overlaps the second half's attention compute).

Device pipeline per core (single Tile program):
  P1  qkv projection (x @ W_qkv.T) for the local 12 feature blocks (PE)
  P2  RMSNorm (over dh=64) + RoPE on q,k in natural [token, feat] layout,
      then PE-transpose q,k into [dh, token] layout
  P3  per (query-half, head, key-block): scoresT = K_j Q^T on PE, exp on ACT
      (scale=1/8 folded in; no max-subtraction -- |score/8| <= 8 because q,k
      are RMS-normalized), PV accumulation with a ones-row appended to V so
      the softmax denominator falls out of the same matmul
  P4  (per half) normalize by 1/den, out-projection partials, ReduceScatter

The block-causal mask (frames of 128 = tile size) is handled by loop bounds;
the single irregular exclusion (last query frame, first key frame) is handled
by zeroing those probs before the PV matmul.

Matmul operands are cast to MM_DTYPE (bf16 by default: fp32/fp32r matmuls run
2-3x slower per row and their 4-byte LDWEIGHTS can't use fast-weight-load).
Accumulation stays fp32 in PSUM; softmax/statistics math stays fp32.
"""

import numpy as np

import concourse.bass as bass
from concourse import bacc
import concourse.mybir as mybir
import concourse.tile as tile
from concourse.masks import make_identity

F32 = mybir.dt.float32
F32R = mybir.dt.float32r
BF16 = mybir.dt.bfloat16

B, L, D = 2, 2048, 1024
H, DH = 16, 64
NT = L // 128        # 16 token tiles
HPC = 4              # heads per core
N_CORES = 8
GROUPS = [[0, 1, 2, 3], [4, 5, 6, 7]]
ROPE_THETA = 10000.0
EPS = 1e-6

FQK = 512            # q+k feature columns per core
FV = 256
FTOT = 768

MM_DTYPES = {"bf16": BF16, "f32r": F32R, "f32": F32}


def build_program(mm_dtype="bf16", apply_gamma=False, qkv_bias=False):
    MMDT = MM_DTYPES[mm_dtype]
    nc = bacc.Bacc(num_devices=N_CORES)

    xT = nc.declare_dram_parameter("xT", [D, L], MMDT, isOutput=False)
    wq = nc.declare_dram_parameter("wq", [D, FTOT], MMDT, isOutput=False)
    wo = nc.declare_dram_parameter("wo", [D, D], MMDT, isOutput=False)
    qoff = nc.declare_dram_parameter("qoff", [1, 1], mybir.dt.uint32, isOutput=False)
    cosb = nc.declare_dram_parameter("cosb", [128, NT, 8, 32], F32, isOutput=False)
    sinb = nc.declare_dram_parameter("sinb", [128, NT, 8, 32], F32, isOutput=False)
    if apply_gamma:
        gam = nc.declare_dram_parameter("gam", [8, DH], F32, isOutput=False)
    if qkv_bias:
        bqk = nc.declare_dram_parameter("bqk", [FTOT], F32, isOutput=False)
    # rows [256*half + r] = my shard of query rows [1024*half + 256*rank + r]
    out = nc.declare_dram_parameter("out", [L // 4, D], F32, isOutput=True)

    with tile.TileContext(nc) as tc:
        with (
            tc.tile_pool(name="singles", bufs=1) as singles,
            tc.tile_pool(name="persist", bufs=1) as persist,
            tc.tile_pool(name="dram", bufs=1, space="DRAM") as dram,
        ):
            ident = singles.tile([128, 128], F32)
            make_identity(nc, ident[:])
            epst = singles.tile([128, 1], F32)
            nc.vector.memset(epst[:], EPS)

            # V with ones column appended: [tok%128, tile, head, 65]
            vsb = persist.tile([128, NT, HPC, DH + 1], MMDT)
            ones_ap = vsb[:, :, :, DH : DH + 1]
            if MMDT == F32R:
                ones_ap = ones_ap.bitcast(F32)
            nc.vector.memset(ones_ap, 1.0)

            # transposed q,k head-pairs: [2*64 feat, L]
            qTs = [persist.tile([128, L], MMDT, tag=f"qTs{i}", name=f"qTs{i}")
                   for i in range(2)]
            kTs = [persist.tile([128, L], MMDT, tag=f"kTs{i}", name=f"kTs{i}")
                   for i in range(2)]

            if apply_gamma:
                gamt = singles.tile([128, 8, DH], F32)
                nc.sync.dma_start(
                    gamt[:],
                    bass.AP(tensor=gam.tensor, offset=gam[:].offset,
                            ap=[[0, 128]] + gam[:].ap))
            if qkv_bias:
                bqkt = singles.tile([128, FTOT], F32)
                nc.sync.dma_start(
                    bqkt[:],
                    bass.AP(tensor=bqk.tensor, offset=bqk[:].offset,
                            ap=[[0, 128]] + bqk[:].ap))

            # qkv weights, split by K-chunk pairs so matmuls can start early
            wqt = []
            wqr = wq[:].rearrange("(kc p) f -> p kc f", p=128)
            for wi in range(4):
                wt = singles.tile([128, 2, FTOT], MMDT, name=f"wqt{wi}")
                nc.sync.dma_start(wt[:], wqr[:, 2 * wi : 2 * wi + 2, :])
                wqt.append(wt)

            attnStack = [persist.tile([128, L], MMDT, tag=f"ast{i}", name=f"ast{i}")
                         for i in range(2)]
            den_d = dram.tile([2, HPC, 1024], F32)      # [half, head, q]
            rden_d = dram.tile([2, HPC, 1024], F32)     # reciprocals
            wos = persist.tile([128, 8, D], MMDT)

            # query-column offset of this core's shard within a half
            qreg = nc.sync.alloc_register("qoff_r")
            nc.sync.reg_load(qreg, qoff[0:1, 0:1])
            qv = nc.sync.snap(qreg, donate=True, min_val=0, max_val=768)

            xTr = xT[:].rearrange("(kc p) l -> p kc l", p=128)

            # ---------------- P1 + P2 (per token-quarter) ----------------
            def emit_p12_quarter(qtr, sbp, psp, merged):
                t0 = qtr * 4
                tok0 = t0 * 128

                xq = sbp.tile([128, 8, 512], MMDT, tag="xq", name=f"xq{qtr}")
                nc.sync.dma_start(xq[:], xTr[:, :, tok0 : tok0 + 512])
                cq = sbp.tile([128, 4, 8, 32], F32, tag="cq", name=f"cq{qtr}")
                nc.sync.dma_start(cq[:], cosb[:, t0 : t0 + 4])
                sq_ = sbp.tile([128, 4, 8, 32], F32, tag="sq_", name=f"sq{qtr}")
                nc.sync.dma_start(sq_[:], sinb[:, t0 : t0 + 4])

                qkraw = sbp.tile([128, 4, 8, DH], F32, tag="qkraw",
                                 name=f"qkraw{qtr}")

                for t4 in range(4):
                    t = t0 + t4
                    if merged:
                        qv_ps = psp.tile([128, FTOT], F32, tag="qk_ps",
                                         name=f"qv{qtr}_{t4}")
                        qk_ps = qv_ps[:, 0:FQK]
                        v_ps = qv_ps[:, FQK:FTOT]
                    else:
                        qk_t = psp.tile([128, FQK], F32, tag="qk_ps",
                                        name=f"qk{qtr}_{t4}")
                        v_t = psp.tile([128, FV], F32, tag="v_ps",
                                       name=f"v{qtr}_{t4}")
                        qk_ps = qk_t[:]
                        v_ps = v_t[:]
                    for kc in range(8):
                        lhsT = xq[:, kc, t4 * 128 : t4 * 128 + 128]
                        wv = wqt[kc // 2][:, kc % 2]
                        nc.tensor.matmul(
                            qk_ps, lhsT, wv[:, 0:FQK],
                            start=(kc == 0), stop=(kc == 7))
                        nc.tensor.matmul(
                            v_ps, lhsT, wv[:, FQK:FTOT],
                            start=(kc == 0), stop=(kc == 7))
                    if qkv_bias:
                        nc.vector.tensor_add(
                            qkraw[:, t4],
                            qk_ps.rearrange("p (g d) -> p g d", d=DH),
                            bqkt[:, 0:FQK].rearrange("p (g d) -> p g d", d=DH))
                        nc.vector.tensor_add(
                            vsb[:, t, :, 0:DH],
                            v_ps.rearrange("p (g d) -> p g d", d=DH),
                            bqkt[:, FQK:FTOT].rearrange("p (g d) -> p g d", d=DH))
                    else:
                        nc.scalar.copy(
                            qkraw[:, t4],
                            qk_ps.rearrange("p (g d) -> p g d", d=DH))
                        nc.scalar.copy(
                            vsb[:, t, :, 0:DH],
                            v_ps.rearrange("p (g d) -> p g d", d=DH))

                if apply_gamma:
                    gview = bass.AP(
                        tensor=gamt.tensor, offset=gamt[:].offset,
                        ap=[gamt[:].ap[0], [0, 4]] + gamt[:].ap[1:])
                    nc.vector.tensor_mul(qkraw[:], qkraw[:], gview)

                # RMS statistics
                sqt = sbp.tile([128, 4, 8, DH], F32, tag="qkrot",
                               name=f"sqt{qtr}")
                nc.vector.tensor_mul(sqt[:], qkraw[:], qkraw[:])
                ssq = sbp.tile([128, 4, 8], F32, tag="ssq", name=f"ssq{qtr}")
                nc.vector.reduce_sum(ssq[:], sqt[:], axis=mybir.AxisListType.X)
                nc.scalar.activation(
                    ssq[:], ssq[:], mybir.ActivationFunctionType.Sqrt,
                    bias=epst[:], scale=1.0 / DH)
                nc.vector.reciprocal(ssq[:], ssq[:])

                # RoPE
                qkrot = sbp.tile([128, 4, 8, DH], F32, tag="qkrot",
                                 name=f"qkrot{qtr}")
                q1 = qkraw[:, :, :, 0:32]
                q2 = qkraw[:, :, :, 32:64]
                mA = sbp.tile([128, 4, 8, 32], F32, tag="mA", name=f"mA{qtr}")
                mB = sbp.tile([128, 4, 8, 32], F32, tag="mB", name=f"mB{qtr}")
                nc.vector.tensor_mul(mA[:], q1, cq[:])
                nc.vector.tensor_mul(mB[:], q2, sq_[:])
                nc.vector.tensor_sub(qkrot[:, :, :, 0:32], mA[:], mB[:])
                mC = sbp.tile([128, 4, 8, 32], F32, tag="mA", name=f"mC{qtr}")
                mD = sbp.tile([128, 4, 8, 32], F32, tag="mB", name=f"mD{qtr}")
                nc.vector.tensor_mul(mC[:], q2, cq[:])
                nc.vector.tensor_mul(mD[:], q1, sq_[:])
                nc.vector.tensor_add(qkrot[:, :, :, 32:64], mC[:], mD[:])

                # apply 1/rms (broadcast [128,4,8] over dh)
                rview = bass.AP(
                    tensor=ssq.tensor, offset=ssq[:].offset,
                    ap=ssq[:].ap + [[0, DH]])
                nc.vector.tensor_mul(qkrot[:], qkrot[:], rview)

                # transpose pairs into qTs/kTs (f32 -> psum, cast on copy)
                for t4 in range(4):
                    t = t0 + t4
                    for pr in range(4):
                        tp = psp.tile([128, 128], F32,
                                      tag="qk_ps" if merged else "tp",
                                      name=f"tp{qtr}_{t4}_{pr}")
                        nc.tensor.transpose(
                            tp[:], qkrot[:, t4, 2 * pr : 2 * pr + 2, :],
                            ident[:])
                        dst = (qTs if pr < 2 else kTs)[pr % 2]
                        nc.vector.tensor_copy(
                            dst[:, t * 128 : (t + 1) * 128], tp[:])

            # ---------------- P3: one head of one query half ----------------
            def emit_p3_head(half, h, scps, atps, pbsb, recp):
                qlo = half * 1024
                jmax = 8 if half == 0 else 16
                kt = kTs[h // 2]
                qt = qTs[h // 2]
                pb0 = (h % 2) * 64
                at = atps.tile([DH + 1, 1024], F32, tag="at",
                               name=f"at{half}_{h}")
                for j in range(jmax):
                    wlo = max(j * 128, qlo)     # global query col start
                    w = qlo + 1024 - wlo
                    sc = scps.tile([128, 1024], F32, tag="sc",
                                   name=f"sc{half}_{h}_{j}")
                    for c0 in range(0, w, 512):
                        cw = min(512, w - c0)
                        nc.tensor.matmul(
                            sc[:, c0 : c0 + cw],
                            kt[pb0 : pb0 + 64, j * 128 : (j + 1) * 128],
                            qt[pb0 : pb0 + 64, wlo + c0 : wlo + c0 + cw],
                            start=True, stop=True)
                    pb = pbsb.tile([128, 1024], MMDT, tag="pb",
                                   name=f"pb{half}_{h}_{j}")
                    nc.scalar.activation(
                        pb[:, 0:w], sc[:, 0:w],
                        mybir.ActivationFunctionType.Exp, scale=1.0 / 8.0)
                    if half == 1 and j == 0:
                        # mask: last query frame can't see key frame 0
                        mask_ap = pb[:, 896:1024]
                        if MMDT == F32R:
                            mask_ap = mask_ap.bitcast(F32)
                        nc.vector.memset(mask_ap, 0.0)
                    s_rel = wlo - qlo           # window start within half
                    for b0 in range(0, 1024, 512):
                        seg0 = max(s_rel, b0)
                        seg1 = b0 + 512
                        if seg0 >= seg1:
                            continue
                        nc.tensor.matmul(
                            at[:, seg0:seg1],
                            vsb[:, j, h, :],
                            pb[:, seg0 - s_rel : seg1 - s_rel],
                            start=(j == 0),
                            stop=(j == jmax - 1
                                  or (j + 1) * 128 >= qlo + seg1))
                nc.vector.tensor_copy(
                    attnStack[h // 2][pb0 : pb0 + 64, qlo : qlo + 1024],
                    at[0:DH, :])
                # stash the (un-reciprocated) denominator row
                rec = recp.tile([DH + 1, 1024], F32, tag="rec",
                                name=f"rec{half}_{h}")
                nc.vector.tensor_copy(rec[DH : DH + 1, :],
                                      at[DH : DH + 1, :])
                nc.gpsimd.dma_start(den_d[half, h, :], rec[DH : DH + 1, :])

            # -------- normalization + AllGather (per head-pair) ----------
            def emit_norm_ag(half, pr, recp, denbp):
                # reciprocal of the two heads' denominators on a [128,16] repack
                dpak = recp.tile([128, 16], F32, tag="dpak",
                                 name=f"dpak{half}_{pr}")
                dh_ap = den_d[half, 2 * pr : 2 * pr + 2]
                nc.gpsimd.dma_start(
                    dpak[:],
                    bass.AP(tensor=dh_ap.tensor, offset=dh_ap.offset,
                            ap=[[16, 128], [1, 16]]))
                nc.vector.reciprocal(dpak[:], dpak[:])
                rh_ap = rden_d[half, 2 * pr : 2 * pr + 2]
                nc.gpsimd.dma_start(
                    bass.AP(tensor=rh_ap.tensor, offset=rh_ap.offset,
                            ap=[[16, 128], [1, 16]]),
                    dpak[:])

                qlo = half * 1024
                denb = denbp.tile([128, 1024], F32, tag="denb",
                                  name=f"denb{half}_{pr}")
                for s in range(2):
                    src_ap = rden_d[half, 2 * pr + s, :]
                    nc.gpsimd.dma_start(
                        denb[s * 64 : (s + 1) * 64, :],
                        bass.AP(tensor=src_ap.tensor, offset=src_ap.offset,
                                ap=[[0, 64]] + src_ap.ap))
                nc.vector.tensor_mul(
                    attnStack[pr][:, qlo : qlo + 1024],
                    attnStack[pr][:, qlo : qlo + 1024],
                    denb[:])

                ag_in = dram.tile([128, 1024], MMDT, name=f"agin{half}_{pr}")
                nc.sync.dma_start(ag_in[:], attnStack[pr][:, qlo : qlo + 1024])
                ago = dram.tile([4, 128, 1024], MMDT, name=f"ago{half}_{pr}")
                nc.gpsimd.collective_compute(
                    "AllGather", mybir.AluOpType.bypass,
                    replica_groups=GROUPS,
                    ins=[ag_in[:].opt()], outs=[ago[:].opt()])
                return ago

            def emit_p4(half, agos, agp, osb, scps):
                agsb = []
                for pr in range(2):
                    a = agp.tile([128, 4, 256], MMDT, tag=f"agsb{pr}",
                                 name=f"agsb{half}_{pr}")
                    agov = agos[pr][:].rearrange("s p q -> p s q")
                    nc.sync.dma_start(a[:], agov[:, :, bass.ds(qv, 256)])
                    agsb.append(a)
                for qt in range(2):
                    ost = osb.tile([128, D], F32, tag="ost",
                                   name=f"ost{half}_{qt}")
                    for o0 in range(0, D, 512):
                        op = scps.tile([128, 512], F32, tag="sc",
                                       name=f"op{half}_{qt}_{o0}")
                        for pr in range(2):
                            for s in range(4):
                                nc.tensor.matmul(
                                    op[:],
                                    agsb[pr][:, s, qt * 128 : (qt + 1) * 128],
                                    wos[:, s * 2 + pr, o0 : o0 + 512],
                                    start=(pr == 0 and s == 0),
                                    stop=(pr == 1 and s == 3))
                        nc.vector.tensor_copy(ost[:, o0 : o0 + 512], op[:])
                    nc.sync.dma_start(
                        out[half * 256 + qt * 128
                            : half * 256 + (qt + 1) * 128, :],
                        ost[:])

            # ---------------- emission schedule ----------------
            with (
                tc.tile_pool(name="p12sbA", bufs=2) as p12sbA,
                tc.tile_pool(name="p12psA", bufs=2, space="PSUM") as p12psA,
            ):
                for _q in range(4):
                    emit_p12_quarter(_q, p12sbA, p12psA, merged=False)

            with (
                tc.tile_pool(name="scps", bufs=3, space="PSUM") as scps,
                tc.tile_pool(name="atps", bufs=1, space="PSUM") as atps,
                tc.tile_pool(name="pbsb", bufs=4) as pbsb,
                tc.tile_pool(name="recp", bufs=2) as recp,
                tc.tile_pool(name="denbp", bufs=2) as denbp,
                tc.tile_pool(name="osb", bufs=3) as osb,
                tc.tile_pool(name="agp", bufs=2) as agp,
            ):
                agos0 = []
                emit_p3_head(0, 0, scps, atps, pbsb, recp)
                emit_p3_head(0, 1, scps, atps, pbsb, recp)
                agos0.append(emit_norm_ag(0, 0, recp, denbp))
                emit_p3_head(0, 2, scps, atps, pbsb, recp)
                emit_p3_head(0, 3, scps, atps, pbsb, recp)
                agos0.append(emit_norm_ag(0, 1, recp, denbp))
                nc.scalar.dma_start(
                    wos[:], wo[:].rearrange("(kc p) o -> p kc o", p=128))
                emit_p4(0, agos0, agp, osb, scps)

                agos1 = []
                emit_p3_head(1, 0, scps, atps, pbsb, recp)
                emit_p3_head(1, 1, scps, atps, pbsb, recp)
                agos1.append(emit_norm_ag(1, 0, recp, denbp))
                emit_p3_head(1, 2, scps, atps, pbsb, recp)
                emit_p3_head(1, 3, scps, atps, pbsb, recp)
                agos1.append(emit_norm_ag(1, 1, recp, denbp))
                emit_p4(1, agos1, agp, osb, scps)

    nc.compile()
    return nc


_PROG_CACHE = {}


def _get_program(key):
    if key not in _PROG_CACHE:
        _PROG_CACHE[key] = build_program(*key)
    return _PROG_CACHE[key]


def _host_inputs(x, W_qkv, b_qkv, W_out, b_out, q_gamma, k_gamma,
                 mm_dtype="bf16"):
    if mm_dtype == "bf16":
        import ml_dtypes
        mmnp = ml_dtypes.bfloat16
    else:
        mmnp = np.float32
    x = np.asarray(x, dtype=np.float32)
    W_qkv = np.asarray(W_qkv, dtype=np.float32)
    b_qkv = np.asarray(b_qkv, dtype=np.float32)
    W_out = np.asarray(W_out, dtype=np.float32)
    q_gamma = np.asarray(q_gamma, dtype=np.float32)
    k_gamma = np.asarray(k_gamma, dtype=np.float32)

    apply_gamma = not (np.all(q_gamma == 1.0) and np.all(k_gamma == 1.0))
    qkv_bias = bool(np.any(b_qkv))

    # rope tables: pos = t*128 + p
    pos = np.arange(L, dtype=np.float64).reshape(NT, 128).T  # [128, NT]
    inv = 1.0 / (ROPE_THETA ** (np.arange(32, dtype=np.float64) / 32.0))
    ang = pos[:, :, None] * inv[None, None, :]               # [128, NT, 32]
    cosb = np.broadcast_to(
        np.cos(ang)[:, :, None, :], (128, NT, 8, 32)).astype(np.float32).copy()
    sinb = np.broadcast_to(
        np.sin(ang)[:, :, None, :], (128, NT, 8, 32)).astype(np.float32).copy()

    Wq = W_qkv[0 * D : 1 * D]
    Wk = W_qkv[1 * D : 2 * D]
    Wv = W_qkv[2 * D : 3 * D]
    WoT = np.ascontiguousarray(W_out.T)  # [d_in, d_out]

    in_maps = []
    for c in range(N_CORES):
        b = c // 4
        h0 = 4 * (c % 4)
        rows = slice(h0 * DH, (h0 + HPC) * DH)
        wq_c = np.ascontiguousarray(
            np.concatenate([Wq[rows], Wk[rows], Wv[rows]], axis=0).T)
        m = {
            "xT": np.ascontiguousarray(x[b].T).astype(mmnp),
            "wq": wq_c.astype(mmnp),
            "wo": WoT.astype(mmnp),
            "cosb": cosb,
            "sinb": sinb,
            "qoff": np.array([[256 * (c % 4)]], dtype=np.uint32),
        }
        if apply_gamma:
            m["gam"] = np.ascontiguousarray(
                np.concatenate([np.broadcast_to(q_gamma, (4, DH)),
                                np.broadcast_to(k_gamma, (4, DH))], axis=0))
        if qkv_bias:
            m["bqk"] = np.ascontiguousarray(np.concatenate(
                [b_qkv[0 * D : 1 * D][rows], b_qkv[1 * D : 2 * D][rows],
                 b_qkv[2 * D : 3 * D][rows]]))
        in_maps.append(m)

    key = (mm_dtype, apply_gamma, qkv_bias)
    return key, in_maps


def _assemble(results, b_out):
    y = np.empty((B, L, D), dtype=np.float32)
    for c in range(N_CORES):
        b = c // 4
        r = c % 4
        o = results[c]["out"]
        for half in range(2):
            rows = slice(1024 * half + 256 * r, 1024 * half + 256 * r + 256)
            y[b, rows, :] = o[256 * half : 256 * half + 256]
    b_out = np.asarray(b_out, dtype=np.float32)
    if np.any(b_out):
        y += b_out
    return y


def _install_ntff_hook():
    """Register the axon NTFF profiling hook (the container's antenv stub
    lacks axon_hooks; replicate what trn_boot would have registered)."""
    import sys
    import types
    try:
        from antenv.axon_hooks import get_axon_ntff_profile_hook  # noqa: F401
        return
    except ImportError:
        pass
    try:
        from trn_agent_boot.trn_boot import _ntff_profile_via_ctypes
        hook = _ntff_profile_via_ctypes("/opt/axon/libaxon_pjrt.so")
    except Exception:
        hook = None
    import antenv
    mod = types.ModuleType("antenv.axon_hooks")
    mod.get_axon_ntff_profile_hook = lambda: hook
    mod.set_axon_ntff_profile_hook = lambda h: None
    antenv.axon_hooks = mod
    sys.modules["antenv.axon_hooks"] = mod


def kernel(x, W_qkv, b_qkv, W_out, b_out, q_gamma, k_gamma, _trace=False,
           _mm_dtype="bf16"):
    from concourse.bass_utils import run_bass_kernel_spmd
    if _trace:
        _install_ntff_hook()

    key, in_maps = _host_inputs(x, W_qkv, b_qkv, W_out, b_out,
                                q_gamma, k_gamma, _mm_dtype)
    nc = _get_program(key)
    res = run_bass_kernel_spmd(nc, in_maps, core_ids=list(range(N_CORES)),
                               trace=_trace,
                               trace_cores=list(range(N_CORES)) if _trace else None)
    y = _assemble(res.results, b_out)
    if _trace:
        return y, res
    return y



# revision 8
# speedup vs baseline: 1.1436x; 1.1436x over previous
"""Block-causal sparse attention (QKNorm + RoPE) for Trainium2, 8 NeuronCores.

Sharding: batch*head parallel. 2 batches x 16 heads = 32 (b,h) pairs; core c
handles batch c//4, heads 4*(c%4) .. 4*(c%4)+4. The out-projection needs all
16 heads' attention channels, so each 4-core batch group exchanges attention
outputs with an AllToAll (each core keeps only its 256-query slice of every
peer's channels -- 4x less traffic than an AllGather) and then computes the
out-projection rows it owns.

Device pipeline per core (single Tile program):
  P1  qkv projection (x @ W_qkv.T) for the local 12 feature blocks (PE),
      PSUM drained by the scalar engine as bf16 (plus a Square pass for the
      RMS statistics)
  P2  RMSNorm (over dh=64) + RoPE on q,k in natural [token, feat] layout in
      bf16 on the vector engine, then PE-transpose (bf16, 1 cyc/row) q,k into
      [dh, token] layout
  P3  per (query-half, head, key-block): scoresT = K_j Q^T on PE, exp on ACT
      (scale=1/8 folded in; no max-subtraction -- |score/8| <= 8 because q,k
      are RMS-normalized), PV accumulation with a ones-row appended to V so
      the softmax denominator falls out of the same matmul. The denominator
      reciprocal is broadcast with one SBUF->SBUF DMA and fused into the
      PSUM->attnStack move.
  P4  out-projection partials from the AllToAll results.

Emission order keeps the in-order tensor queue busy across the collectives:
all eight P3 head-blocks are emitted before the first out-projection, so the
AllToAlls for query-half 0 complete while half 1's attention still computes.

The block-causal mask (frames of 128 = tile size) is handled by loop bounds;
the single irregular exclusion (last query frame, first key frame) is handled
by zeroing those probs before the PV matmul.

Matmul operands are bf16 (fp32 matmuls run 4x slower per row); accumulation
stays fp32 in PSUM; softmax statistics stay fp32.
"""

import numpy as np

import concourse.bass as bass
from concourse import bacc
import concourse.mybir as mybir
import concourse.tile as tile
from concourse.masks import make_identity

F32 = mybir.dt.float32
BF16 = mybir.dt.bfloat16

B, L, D = 2, 2048, 1024
H, DH = 16, 64
NT = L // 128        # 16 token tiles
HPC = 4              # heads per core
N_CORES = 8
GROUPS = [[0, 1, 2, 3], [4, 5, 6, 7]]
ROPE_THETA = 10000.0
EPS = 1e-6

FQK = 512            # q+k feature columns per core
FV = 256
FTOT = 768


def build_program(apply_gamma=False, qkv_bias=False):
    nc = bacc.Bacc(num_devices=N_CORES)

    xT = nc.declare_dram_parameter("xT", [D, L], BF16, isOutput=False)
    wq = nc.declare_dram_parameter("wq", [D, FTOT], BF16, isOutput=False)
    wo = nc.declare_dram_parameter("wo", [D, D], BF16, isOutput=False)
    cosb = nc.declare_dram_parameter("cosb", [128, NT, 32], BF16, isOutput=False)
    sinb = nc.declare_dram_parameter("sinb", [128, NT, 32], BF16, isOutput=False)
    if apply_gamma:
        gam = nc.declare_dram_parameter("gam", [8, DH], F32, isOutput=False)
    if qkv_bias:
        bqk = nc.declare_dram_parameter("bqk", [FTOT], F32, isOutput=False)
    qoff = nc.declare_dram_parameter("qoff", [1, 1], mybir.dt.uint32, isOutput=False)
    # rows [256*half + r] = my shard of query rows [1024*half + 256*rank + r]
    out = nc.declare_dram_parameter("out", [L // 4, D], F32, isOutput=True)

    with tile.TileContext(nc) as tc:
        with (
            tc.tile_pool(name="singles", bufs=1) as singles,
            tc.tile_pool(name="persist", bufs=1) as persist,
            tc.tile_pool(name="dram", bufs=1, space="DRAM") as dram,
        ):
            ident = singles.tile([128, 128], BF16)
            make_identity(nc, ident[:])
            epst = singles.tile([128, 1], F32)
            nc.vector.memset(epst[:], EPS)

            # V with ones column appended: [tok%128, tile, head, 65]
            vsb = persist.tile([128, NT, HPC, DH + 1], BF16)
            nc.vector.memset(vsb[:, :, :, DH : DH + 1], 1.0)

            # transposed q,k head-pairs: [2*64 feat, L]
            qTs = [persist.tile([128, L], BF16, tag=f"qTs{i}", name=f"qTs{i}")
                   for i in range(2)]
            kTs = [persist.tile([128, L], BF16, tag=f"kTs{i}", name=f"kTs{i}")
                   for i in range(2)]

            # rope tables, compact (head dim broadcast via SBUF DMA below)
            cost = singles.tile([128, NT, 32], BF16)
            nc.sync.dma_start(cost[:], cosb[:])
            sint = singles.tile([128, NT, 32], BF16)
            nc.sync.dma_start(sint[:], sinb[:])
            # replicated over the 8 q/k heads: [128, NT, 8, 32]
            cos8 = persist.tile([128, NT, 8, 32], BF16)
            sin8 = persist.tile([128, NT, 8, 32], BF16)
            for g in range(8):
                nc.sync.dma_start(cos8[:, :, g, :], cost[:])
                nc.sync.dma_start(sin8[:, :, g, :], sint[:])

            if apply_gamma:
                gamt = singles.tile([128, 8, DH], F32)
                nc.sync.dma_start(
                    gamt[:],
                    bass.AP(tensor=gam.tensor, offset=gam[:].offset,
                            ap=[[0, 128]] + gam[:].ap))
            if qkv_bias:
                bqkt = singles.tile([128, FTOT], F32)
                nc.sync.dma_start(
                    bqkt[:],
                    bass.AP(tensor=bqk.tensor, offset=bqk[:].offset,
                            ap=[[0, 128]] + bqk[:].ap))

            # qkv weights, split by K-chunk pairs so matmuls can start early
            wqt = []
            wqr = wq[:].rearrange("(kc p) f -> p kc f", p=128)
            for wi in range(4):
                wt = singles.tile([128, 2, FTOT], BF16, name=f"wqt{wi}")
                nc.sync.dma_start(wt[:], wqr[:, 2 * wi : 2 * wi + 2, :])
                wqt.append(wt)

            attnStack = [persist.tile([128, L], BF16, tag=f"ast{i}", name=f"ast{i}")
                         for i in range(2)]
            wos = persist.tile([128, 8, D], BF16)

            # query-column offset of this core's shard within a half
            qreg = nc.sync.alloc_register("qoff_r")
            nc.sync.reg_load(qreg, qoff[0:1, 0:1])
            qv = nc.sync.snap(qreg, donate=True, min_val=0, max_val=768)

            xTr = xT[:].rearrange("(kc p) l -> p kc l", p=128)

            # ---------------- P1: qkv matmuls for one token-quarter ----------
            def emit_p1_quarter(qtr, sbp, psp):
                t0 = qtr * 4
                tok0 = t0 * 128

                xqA = sbp.tile([128, 4, 512], BF16, tag="xqA", name=f"xqA{qtr}")
                nc.sync.dma_start(xqA[:], xTr[:, 0:4, tok0 : tok0 + 512])
                xqB = sbp.tile([128, 4, 512], BF16, tag="xqB", name=f"xqB{qtr}")
                nc.sync.dma_start(xqB[:], xTr[:, 4:8, tok0 : tok0 + 512])

                qkraw = sbp.tile([128, 4, 8, DH], BF16, tag="qkraw",
                                 name=f"qkraw{qtr}")
                sqt = sbp.tile([128, 4, 8, DH], BF16, tag="sqt",
                               name=f"sqt{qtr}")

                for t4 in range(4):
                    t = t0 + t4
                    qv_ps = psp.tile([128, FTOT], F32, tag="qv_ps",
                                     name=f"qv{qtr}_{t4}")
                    qk_ps = qv_ps[:, 0:FQK]
                    v_ps = qv_ps[:, FQK:FTOT]
                    for kc in range(8):
                        lhsT = (xqA if kc < 4 else xqB)[
                            :, kc % 4, t4 * 128 : t4 * 128 + 128]
                        wv = wqt[kc // 2][:, kc % 2]
                        nc.tensor.matmul(
                            qk_ps, lhsT, wv[:, 0:FQK],
                            start=(kc == 0), stop=(kc == 7))
                        nc.tensor.matmul(
                            v_ps, lhsT, wv[:, FQK:FTOT],
                            start=(kc == 0), stop=(kc == 7))
                    if qkv_bias:
                        nc.vector.tensor_add(
                            qkraw[:, t4],
                            qk_ps.rearrange("p (g d) -> p g d", d=DH),
                            bqkt[:, 0:FQK].rearrange("p (g d) -> p g d", d=DH))
                        nc.vector.tensor_add(
                            vsb[:, t, :, 0:DH],
                            v_ps.rearrange("p (g d) -> p g d", d=DH),
                            bqkt[:, FQK:FTOT].rearrange("p (g d) -> p g d", d=DH))
                        nc.scalar.activation(
                            sqt[:, t4].rearrange("p g d -> p (g d)"),
                            qkraw[:, t4].rearrange("p g d -> p (g d)"),
                            mybir.ActivationFunctionType.Square)
                    else:
                        nc.scalar.copy(
                            qkraw[:, t4],
                            qk_ps.rearrange("p (g d) -> p g d", d=DH))
                        nc.scalar.copy(
                            vsb[:, t, :, 0:DH],
                            v_ps.rearrange("p (g d) -> p g d", d=DH))
                        nc.scalar.activation(
                            sqt[:, t4].rearrange("p g d -> p (g d)"),
                            qk_ps, mybir.ActivationFunctionType.Square)
                return qkraw, sqt

            # ---------------- P2: rms + rope + transpose for a quarter -------
            def emit_p2_quarter(qtr, qkraw, sqt, sbp, psp):
                t0 = qtr * 4

                # RMS statistics (rope is norm-preserving, so stats from raw)
                ssq = sbp.tile([128, 4, 8], F32, tag="ssq", name=f"ssq{qtr}")
                nc.vector.reduce_sum(ssq[:], sqt[:], axis=mybir.AxisListType.X)
                nc.scalar.activation(
                    ssq[:], ssq[:], mybir.ActivationFunctionType.Sqrt,
                    bias=epst[:], scale=1.0 / DH)
                nc.vector.reciprocal(ssq[:], ssq[:])
                rmsb = sbp.tile([128, 4, 8], BF16, tag="rmsb", name=f"rmsb{qtr}")
                nc.vector.tensor_copy(rmsb[:], ssq[:])

                if apply_gamma:
                    gview = bass.AP(
                        tensor=gamt.tensor, offset=gamt[:].offset,
                        ap=[gamt[:].ap[0], [0, 4]] + gamt[:].ap[1:])
                    nc.vector.tensor_mul(qkraw[:], qkraw[:], gview)

                # RoPE (all bf16)
                cq = cos8[:, t0 : t0 + 4]
                sq_ = sin8[:, t0 : t0 + 4]
                qkrot = sbp.tile([128, 4, 8, DH], BF16, tag="qkrot",
                                 name=f"qkrot{qtr}")
                q1 = qkraw[:, :, :, 0:32]
                q2 = qkraw[:, :, :, 32:64]
                mA = sbp.tile([128, 4, 8, 32], BF16, tag="mA", name=f"mA{qtr}")
                mB = sbp.tile([128, 4, 8, 32], BF16, tag="mB", name=f"mB{qtr}")
                nc.vector.tensor_mul(mA[:], q1, cq)
                nc.vector.tensor_mul(mB[:], q2, sq_)
                nc.vector.tensor_sub(qkrot[:, :, :, 0:32], mA[:], mB[:])
                mC = sbp.tile([128, 4, 8, 32], BF16, tag="mA", name=f"mC{qtr}")
                mD = sbp.tile([128, 4, 8, 32], BF16, tag="mB", name=f"mD{qtr}")
                nc.vector.tensor_mul(mC[:], q2, cq)
                nc.vector.tensor_mul(mD[:], q1, sq_)
                nc.vector.tensor_add(qkrot[:, :, :, 32:64], mC[:], mD[:])

                # apply 1/rms (broadcast [128,4,8] over dh)
                rview = bass.AP(
                    tensor=rmsb.tensor, offset=rmsb[:].offset,
                    ap=rmsb[:].ap + [[0, DH]])
                nc.vector.tensor_mul(qkrot[:], qkrot[:], rview)

                # transpose pairs into qTs/kTs (bf16 psum, ACT copies out)
                for t4 in range(4):
                    t = t0 + t4
                    for pr in range(4):
                        tp = psp.tile([128, 128], BF16, tag="tp",
                                      name=f"tp{qtr}_{t4}_{pr}")
                        nc.tensor.transpose(
                            tp[:], qkrot[:, t4, 2 * pr : 2 * pr + 2, :],
                            ident[:])
                        dst = (qTs if pr < 2 else kTs)[pr % 2]
                        nc.scalar.copy(
                            dst[:, t * 128 : (t + 1) * 128], tp[:])

            # ---------------- P3: one head of one query half ----------------
            def emit_p3_head(half, h, scps, atps, pbsb, recp, denbp):
                qlo = half * 1024
                jmax = 8 if half == 0 else 16
                kt = kTs[h // 2]
                qt = qTs[h // 2]
                pb0 = (h % 2) * 64
                at = atps.tile([DH + 1, 1024], F32, tag="at",
                               name=f"at{half}_{h}")
                for j in range(jmax):
                    wlo = max(j * 128, qlo)     # global query col start
                    w = qlo + 1024 - wlo
                    sc = scps.tile([128, 1024], F32, tag="sc",
                                   name=f"sc{half}_{h}_{j}")
                    for c0 in range(0, w, 512):
                        cw = min(512, w - c0)
                        nc.tensor.matmul(
                            sc[:, c0 : c0 + cw],
                            kt[pb0 : pb0 + 64, j * 128 : (j + 1) * 128],
                            qt[pb0 : pb0 + 64, wlo + c0 : wlo + c0 + cw],
                            start=True, stop=True)
                    pb = pbsb.tile([128, 1024], BF16, tag="pb",
                                   name=f"pb{half}_{h}_{j}")
                    nc.scalar.activation(
                        pb[:, 0:w], sc[:, 0:w],
                        mybir.ActivationFunctionType.Exp, scale=1.0 / 8.0)
                    if half == 1 and j == 0:
                        # mask: last query frame can't see key frame 0
                        nc.vector.memset(pb[:, 896:1024], 0.0)
                    s_rel = wlo - qlo           # window start within half
                    for b0 in range(0, 1024, 512):
                        seg0 = max(s_rel, b0)
                        seg1 = b0 + 512
                        if seg0 >= seg1:
                            continue
                        nc.tensor.matmul(
                            at[:, seg0:seg1],
                            vsb[:, j, h, :],
                            pb[:, seg0 - s_rel : seg1 - s_rel],
                            start=(j == 0),
                            stop=(j == jmax - 1
                                  or (j + 1) * 128 >= qlo + seg1))
                # denominator reciprocal, broadcast over the head's 64 rows,
                # fused into the PSUM->attnStack move
                rden = recp.tile([1, 1024], F32, tag="rden",
                                 name=f"rden{half}_{h}")
                nc.vector.reciprocal(rden[:], at[DH : DH + 1, :])
                denb = denbp.tile([64, 1024], F32, tag="denb",
                                  name=f"denb{half}_{h}")
                nc.gpsimd.partition_broadcast(denb[:], rden[:])
                nc.vector.tensor_mul(
                    attnStack[h // 2][pb0 : pb0 + 64, qlo : qlo + 1024],
                    at[0:DH, :], denb[:])

            # -------- AllGather of one head-pair's attention channels -------
            def emit_a2a(half, pr):
                qlo = half * 1024
                ag_in = dram.tile([128, 1024], BF16, name=f"a2i{half}_{pr}")
                nc.gpsimd.dma_start(
                    ag_in[:], attnStack[pr][:, qlo : qlo + 1024])
                ago = dram.tile([4, 128, 1024], BF16, name=f"a2o{half}_{pr}")
                nc.gpsimd.collective_compute(
                    "AllGather", mybir.AluOpType.bypass,
                    replica_groups=GROUPS,
                    ins=[ag_in[:].opt()], outs=[ago[:].opt()])
                return ago

            def emit_p4(half, a2as, agp, osb, scps):
                agsb = []
                for pr in range(2):
                    a = agp.tile([128, 4, 256], BF16, tag=f"agsb{pr}",
                                 name=f"agsb{half}_{pr}")
                    agov = a2as[pr][:].rearrange("s p q -> p s q")
                    nc.sync.dma_start(a[:], agov[:, :, bass.ds(qv, 256)])
                    agsb.append(a)
                for qt in range(2):
                    ost = osb.tile([128, D], F32, tag="ost",
                                   name=f"ost{half}_{qt}")
                    for o0 in range(0, D, 512):
                        op = scps.tile([128, 512], F32, tag="sc",
                                       name=f"op{half}_{qt}_{o0}")
                        for pr in range(2):
                            for s in range(4):
                                nc.tensor.matmul(
                                    op[:],
                                    agsb[pr][:, s, qt * 128 : (qt + 1) * 128],
                                    wos[:, s * 2 + pr, o0 : o0 + 512],
                                    start=(pr == 0 and s == 0),
                                    stop=(pr == 1 and s == 3))
                        nc.scalar.copy(ost[:, o0 : o0 + 512], op[:])
                    nc.sync.dma_start(
                        out[half * 256 + qt * 128
                            : half * 256 + (qt + 1) * 128, :],
                        ost[:])

            # ---------------- emission schedule ----------------
            # P1 matmuls run ahead of the (vector-bound) P2 of the previous
            # quarter so the tensor queue never waits on rope.
            with (
                tc.tile_pool(name="p12sb", bufs=2) as p12sb,
                tc.tile_pool(name="p12ps", bufs=2, space="PSUM") as p12ps,
                tc.tile_pool(name="tpps", bufs=2, space="PSUM") as tpps,
            ):
                raws = []
                raws.append(emit_p1_quarter(0, p12sb, p12ps))
                raws.append(emit_p1_quarter(1, p12sb, p12ps))
                emit_p2_quarter(0, *raws[0], p12sb, tpps)
                raws.append(emit_p1_quarter(2, p12sb, p12ps))
                emit_p2_quarter(1, *raws[1], p12sb, tpps)
                raws.append(emit_p1_quarter(3, p12sb, p12ps))
                emit_p2_quarter(2, *raws[2], p12sb, tpps)
                emit_p2_quarter(3, *raws[3], p12sb, tpps)

            nc.scalar.dma_start(
                wos[:], wo[:].rearrange("(kc p) o -> p kc o", p=128))

            with (
                tc.tile_pool(name="scps", bufs=3, space="PSUM") as scps,
                tc.tile_pool(name="atps", bufs=1, space="PSUM") as atps,
                tc.tile_pool(name="pbsb", bufs=4) as pbsb,
                tc.tile_pool(name="recp", bufs=2) as recp,
                tc.tile_pool(name="denbp", bufs=2) as denbp,
                tc.tile_pool(name="osb", bufs=3) as osb,
                tc.tile_pool(name="agp", bufs=2) as agp,
            ):
                emit_p3_head(0, 0, scps, atps, pbsb, recp, denbp)
                emit_p3_head(0, 1, scps, atps, pbsb, recp, denbp)
                a2a00 = emit_a2a(0, 0)
                emit_p3_head(0, 2, scps, atps, pbsb, recp, denbp)
                emit_p3_head(0, 3, scps, atps, pbsb, recp, denbp)
                a2a01 = emit_a2a(0, 1)

                emit_p3_head(1, 0, scps, atps, pbsb, recp, denbp)
                emit_p3_head(1, 1, scps, atps, pbsb, recp, denbp)
                a2a10 = emit_a2a(1, 0)
                emit_p3_head(1, 2, scps, atps, pbsb, recp, denbp)
                emit_p4(0, [a2a00, a2a01], agp, osb, scps)
                emit_p3_head(1, 3, scps, atps, pbsb, recp, denbp)
                a2a11 = emit_a2a(1, 1)
                emit_p4(1, [a2a10, a2a11], agp, osb, scps)

    nc.compile()
    return nc


_PROG_CACHE = {}


def _get_program(key):
    if key not in _PROG_CACHE:
        _PROG_CACHE[key] = build_program(*key)
    return _PROG_CACHE[key]


def _host_inputs(x, W_qkv, b_qkv, W_out, b_out, q_gamma, k_gamma):
    import ml_dtypes
    mmnp = ml_dtypes.bfloat16
    x = np.asarray(x, dtype=np.float32)
    W_qkv = np.asarray(W_qkv, dtype=np.float32)
    b_qkv = np.asarray(b_qkv, dtype=np.float32)
    W_out = np.asarray(W_out, dtype=np.float32)
    q_gamma = np.asarray(q_gamma, dtype=np.float32)
    k_gamma = np.asarray(k_gamma, dtype=np.float32)

    apply_gamma = not (np.all(q_gamma == 1.0) and np.all(k_gamma == 1.0))
    qkv_bias = bool(np.any(b_qkv))

    # rope tables: pos = t*128 + p
    pos = np.arange(L, dtype=np.float64).reshape(NT, 128).T  # [128, NT]
    inv = 1.0 / (ROPE_THETA ** (np.arange(32, dtype=np.float64) / 32.0))
    ang = pos[:, :, None] * inv[None, None, :]               # [128, NT, 32]
    cosb = np.cos(ang).astype(mmnp)
    sinb = np.sin(ang).astype(mmnp)

    Wq = W_qkv[0 * D : 1 * D]
    Wk = W_qkv[1 * D : 2 * D]
    Wv = W_qkv[2 * D : 3 * D]
    WoT = np.ascontiguousarray(W_out.T)  # [d_in, d_out]

    in_maps = []
    for c in range(N_CORES):
        b = c // 4
        h0 = 4 * (c % 4)
        rows = slice(h0 * DH, (h0 + HPC) * DH)
        wq_c = np.ascontiguousarray(
            np.concatenate([Wq[rows], Wk[rows], Wv[rows]], axis=0).T)
        m = {
            "xT": np.ascontiguousarray(x[b].T).astype(mmnp),
            "wq": wq_c.astype(mmnp),
            "wo": WoT.astype(mmnp),
            "cosb": cosb,
            "sinb": sinb,
            "qoff": np.array([[256 * (c % 4)]], dtype=np.uint32),
        }
        if apply_gamma:
            m["gam"] = np.ascontiguousarray(
                np.concatenate([np.broadcast_to(q_gamma, (4, DH)),
                                np.broadcast_to(k_gamma, (4, DH))], axis=0))
        if qkv_bias:
            m["bqk"] = np.ascontiguousarray(np.concatenate(
                [b_qkv[0 * D : 1 * D][rows], b_qkv[1 * D : 2 * D][rows],
                 b_qkv[2 * D : 3 * D][rows]]))
        in_maps.append(m)

    key = (apply_gamma, qkv_bias)
    return key, in_maps


def _assemble(results, b_out):
    y = np.empty((B, L, D), dtype=np.float32)
    for c in range(N_CORES):
        b = c // 4
        r = c % 4
        o = results[c]["out"]
        for half in range(2):
            rows = slice(1024 * half + 256 * r, 1024 * half + 256 * r + 256)
            y[b, rows, :] = o[256 * half : 256 * half + 256]
    b_out = np.asarray(b_out, dtype=np.float32)
    if np.any(b_out):
        y += b_out
    return y


def _install_ntff_hook():
    """Register the axon NTFF profiling hook (the container's antenv stub
    lacks axon_hooks; replicate what trn_boot would have registered)."""
    import sys
    import types
    try:
        from antenv.axon_hooks import get_axon_ntff_profile_hook  # noqa: F401
        return
    except ImportError:
        pass
    try:
        from trn_agent_boot.trn_boot import _ntff_profile_via_ctypes
        hook = _ntff_profile_via_ctypes("/opt/axon/libaxon_pjrt.so")
    except Exception:
        hook = None
    import antenv
    mod = types.ModuleType("antenv.axon_hooks")
    mod.get_axon_ntff_profile_hook = lambda: hook
    mod.set_axon_ntff_profile_hook = lambda h: None
    antenv.axon_hooks = mod
    sys.modules["antenv.axon_hooks"] = mod


def kernel(x, W_qkv, b_qkv, W_out, b_out, q_gamma, k_gamma, _trace=False):
    from concourse.bass_utils import run_bass_kernel_spmd
    if _trace:
        _install_ntff_hook()

    key, in_maps = _host_inputs(x, W_qkv, b_qkv, W_out, b_out,
                                q_gamma, k_gamma)
    nc = _get_program(key)
    res = run_bass_kernel_spmd(nc, in_maps, core_ids=list(range(N_CORES)),
                               trace=_trace,
                               trace_cores=list(range(N_CORES)) if _trace else None)
    y = _assemble(res.results, b_out)
    if _trace:
        return y, res
    return y


# revision 11
# speedup vs baseline: 1.3941x; 1.2191x over previous
"""Block-causal sparse attention (QKNorm + RoPE) for Trainium2, 8 NeuronCores.

Sharding: batch*head parallel. 2 batches x 16 heads = 32 (b,h) pairs; core c
handles batch c//4, heads 4*(c%4) .. 4*(c%4)+4. The out-projection needs all
16 heads' attention channels, so each 4-core batch group runs AllGathers of
the normalized attention outputs, one per (query-quarter, head-pair) -- 8
small collectives that pipeline deeply under the remaining attention compute
-- and each core computes the out-projection rows it owns.

Device pipeline per core (single Tile program):
  P1  qkv projection (x @ W_qkv.T) for the local 12 feature blocks (PE);
      PSUM drained as bf16 by ACT (q,k), gpsimd (v), DVE (squares for RMS)
  P2  RMSNorm (over dh=64) + RoPE on q,k in natural [token, feat] layout in
      bf16 on the vector engine, then PE-transpose (bf16, 1 cyc/row) q,k into
      [dh, token] layout
  P3  per (query-quarter, head-pair, key-block): scoresT = K_j Q^T on PE
      (one matmul per head), a single exp over both heads on ACT (scale=1/8
      folded in; no max-subtraction -- |score/8| <= 8 because q,k are
      RMS-normalized), PV accumulation with a ones-row appended to V so the
      softmax denominator falls out of the same matmul. The denominator
      reciprocal (DVE) is partition-broadcast on gpsimd and fused into the
      PSUM->attnStack move (gpsimd).
  P4  out-projection for this core's 128-row slice of each quarter, from the
      AllGather results.

Emission order keeps the in-order tensor queue busy across the collectives:
quarter r's out-projection is emitted during quarter r+1's attention.

The block-causal mask (frames of 128 = tile size) is handled by loop bounds;
the single irregular exclusion (last query frame, first key frame) is handled
by zeroing those probs before the PV matmul.

Matmul operands are bf16 (fp32 matmuls run 4x slower per row); accumulation
stays fp32 in PSUM; softmax statistics stay fp32.
"""

import numpy as np

import concourse.bass as bass
from concourse import bacc
import concourse.mybir as mybir
import concourse.tile as tile
from concourse.masks import make_identity

F32 = mybir.dt.float32
BF16 = mybir.dt.bfloat16

B, L, D = 2, 2048, 1024
H, DH = 16, 64
NT = L // 128        # 16 token tiles
HPC = 4              # heads per core
N_CORES = 8
GROUPS = [[0, 1, 2, 3], [4, 5, 6, 7]]
ROPE_THETA = 10000.0
EPS = 1e-6

FQK = 512            # q+k feature columns per core
FV = 256
FTOT = 768


def build_program(apply_gamma=False, qkv_bias=False):
    nc = bacc.Bacc(num_devices=N_CORES)

    xT = nc.declare_dram_parameter("xT", [D, L], BF16, isOutput=False)
    wq = nc.declare_dram_parameter("wq", [D, FTOT], BF16, isOutput=False)
    wo = nc.declare_dram_parameter("wo", [D, D], BF16, isOutput=False)
    cosb = nc.declare_dram_parameter("cosb", [128, NT, 8, 32], BF16,
                                     isOutput=False)
    sinb = nc.declare_dram_parameter("sinb", [128, NT, 8, 32], BF16,
                                     isOutput=False)
    if apply_gamma:
        gam = nc.declare_dram_parameter("gam", [8, DH], F32, isOutput=False)
    if qkv_bias:
        bqk = nc.declare_dram_parameter("bqk", [FTOT], F32, isOutput=False)
    qoff = nc.declare_dram_parameter("qoff", [1, 1], mybir.dt.uint32,
                                     isOutput=False)
    # rows [128*r + i] = my shard of query rows [512*r + 128*rank + i]
    out = nc.declare_dram_parameter("out", [L // 4, D], F32, isOutput=True)

    with tile.TileContext(nc) as tc:
        with (
            tc.tile_pool(name="singles", bufs=1) as singles,
            tc.tile_pool(name="persist", bufs=1) as persist,
            tc.tile_pool(name="dram", bufs=1, space="DRAM") as dram,
        ):
            ident = singles.tile([128, 128], BF16)
            make_identity(nc, ident[:])
            epst = singles.tile([128, 1], F32)
            nc.vector.memset(epst[:], EPS)

            # V with ones column appended: [tok%128, tile, head, 65]
            vsb = persist.tile([128, NT, HPC, DH + 1], BF16)
            nc.vector.memset(vsb[:, :, :, DH : DH + 1], 1.0)

            # transposed q,k head-pairs: [2*64 feat, L]
            qTs = [persist.tile([128, L], BF16, tag=f"qTs{i}", name=f"qTs{i}")
                   for i in range(2)]
            kTs = [persist.tile([128, L], BF16, tag=f"kTs{i}", name=f"kTs{i}")
                   for i in range(2)]

            # rope tables (pre-replicated over the 8 q/k heads in HBM)
            cos8 = persist.tile([128, NT, 8, 32], BF16)
            nc.sync.dma_start(cos8[:], cosb[:])
            sin8 = persist.tile([128, NT, 8, 32], BF16)
            nc.sync.dma_start(sin8[:], sinb[:])

            if apply_gamma:
                gamt = singles.tile([128, 8, DH], F32)
                nc.sync.dma_start(
                    gamt[:],
                    bass.AP(tensor=gam.tensor, offset=gam[:].offset,
                            ap=[[0, 128]] + gam[:].ap))
            if qkv_bias:
                bqkt = singles.tile([128, FTOT], F32)
                nc.sync.dma_start(
                    bqkt[:],
                    bass.AP(tensor=bqk.tensor, offset=bqk[:].offset,
                            ap=[[0, 128]] + bqk[:].ap))

            # qkv weights, split by K-chunk pairs so matmuls can start early
            wqt = []
            wqr = wq[:].rearrange("(kc p) f -> p kc f", p=128)
            for wi in range(4):
                wt = singles.tile([128, 2, FTOT], BF16, name=f"wqt{wi}")
                nc.sync.dma_start(wt[:], wqr[:, 2 * wi : 2 * wi + 2, :])
                wqt.append(wt)

            attnStack = [persist.tile([128, L], BF16, tag=f"ast{i}", name=f"ast{i}")
                         for i in range(2)]
            wos = persist.tile([128, 8, D], BF16)

            # query-column offset of this core's shard within a quarter
            qreg = nc.sync.alloc_register("qoff_r")
            nc.sync.reg_load(qreg, qoff[0:1, 0:1])
            qv = nc.sync.snap(qreg, donate=True, min_val=0, max_val=384)

            xTr = xT[:].rearrange("(kc p) l -> p kc l", p=128)

            # ---------------- P1: qkv matmuls for one token-quarter ----------
            def emit_p1_quarter(qtr, sbp, psp):
                t0 = qtr * 4
                tok0 = t0 * 128

                xqA = sbp.tile([128, 4, 512], BF16, tag="xqA", name=f"xqA{qtr}")
                nc.sync.dma_start(xqA[:], xTr[:, 0:4, tok0 : tok0 + 512])
                xqB = sbp.tile([128, 4, 512], BF16, tag="xqB", name=f"xqB{qtr}")
                nc.sync.dma_start(xqB[:], xTr[:, 4:8, tok0 : tok0 + 512])

                qkraw = sbp.tile([128, 4, 8, DH], BF16, tag="qkraw",
                                 name=f"qkraw{qtr}")
                sqt = sbp.tile([128, 4, 8, DH], BF16, tag="sqt",
                               name=f"sqt{qtr}")

                for t4 in range(4):
                    t = t0 + t4
                    qv_ps = psp.tile([128, FTOT], F32, tag="qv_ps",
                                     name=f"qv{qtr}_{t4}")
                    qk_ps = qv_ps[:, 0:FQK]
                    v_ps = qv_ps[:, FQK:FTOT]
                    for kc in range(8):
                        lhsT = (xqA if kc < 4 else xqB)[
                            :, kc % 4, t4 * 128 : t4 * 128 + 128]
                        wv = wqt[kc // 2][:, kc % 2]
                        nc.tensor.matmul(
                            qk_ps, lhsT, wv[:, 0:FQK],
                            start=(kc == 0), stop=(kc == 7))
                        nc.tensor.matmul(
                            v_ps, lhsT, wv[:, FQK:FTOT],
                            start=(kc == 0), stop=(kc == 7))
                    if qkv_bias:
                        nc.vector.tensor_add(
                            qkraw[:, t4],
                            qk_ps.rearrange("p (g d) -> p g d", d=DH),
                            bqkt[:, 0:FQK].rearrange("p (g d) -> p g d", d=DH))
                        nc.vector.tensor_add(
                            vsb[:, t, :, 0:DH],
                            v_ps.rearrange("p (g d) -> p g d", d=DH),
                            bqkt[:, FQK:FTOT].rearrange("p (g d) -> p g d", d=DH))
                    else:
                        nc.scalar.copy(
                            qkraw[:, t4],
                            qk_ps.rearrange("p (g d) -> p g d", d=DH))
                        nc.scalar.copy(
                            vsb[:, t, :, 0:DH],
                            v_ps.rearrange("p (g d) -> p g d", d=DH))
                    nc.vector.tensor_mul(
                        sqt[:, t4], qkraw[:, t4], qkraw[:, t4])
                return qkraw, sqt

            # ---------------- P2: rms + rope + transpose for a quarter -------
            def emit_p2_quarter(qtr, qkraw, sqt, sbp, psp):
                t0 = qtr * 4

                # RMS statistics (rope is norm-preserving, so stats from raw)
                ssq = sbp.tile([128, 4, 8], F32, tag="ssq", name=f"ssq{qtr}")
                nc.vector.reduce_sum(ssq[:], sqt[:], axis=mybir.AxisListType.X)
                nc.scalar.activation(
                    ssq[:], ssq[:], mybir.ActivationFunctionType.Sqrt,
                    bias=epst[:], scale=1.0 / DH)
                nc.vector.reciprocal(ssq[:], ssq[:])
                rmsb = sbp.tile([128, 4, 8], BF16, tag="rmsb", name=f"rmsb{qtr}")
                nc.vector.tensor_copy(rmsb[:], ssq[:])

                if apply_gamma:
                    gview = bass.AP(
                        tensor=gamt.tensor, offset=gamt[:].offset,
                        ap=[gamt[:].ap[0], [0, 4]] + gamt[:].ap[1:])
                    nc.vector.tensor_mul(qkraw[:], qkraw[:], gview)

                # RoPE (all bf16)
                cq = cos8[:, t0 : t0 + 4]
                sq_ = sin8[:, t0 : t0 + 4]
                qkrot = sbp.tile([128, 4, 8, DH], BF16, tag="qkrot",
                                 name=f"qkrot{qtr}")
                q1 = qkraw[:, :, :, 0:32]
                q2 = qkraw[:, :, :, 32:64]
                mA = sbp.tile([128, 4, 8, 32], BF16, tag="mA", name=f"mA{qtr}")
                mB = sbp.tile([128, 4, 8, 32], BF16, tag="mB", name=f"mB{qtr}")
                nc.vector.tensor_mul(mA[:], q1, cq)
                nc.vector.tensor_mul(mB[:], q2, sq_)
                nc.vector.tensor_sub(qkrot[:, :, :, 0:32], mA[:], mB[:])
                mC = sbp.tile([128, 4, 8, 32], BF16, tag="mA", name=f"mC{qtr}")
                mD = sbp.tile([128, 4, 8, 32], BF16, tag="mB", name=f"mD{qtr}")
                nc.vector.tensor_mul(mC[:], q2, cq)
                nc.vector.tensor_mul(mD[:], q1, sq_)
                nc.vector.tensor_add(qkrot[:, :, :, 32:64], mC[:], mD[:])

                # apply 1/rms (broadcast [128,4,8] over dh)
                rview = bass.AP(
                    tensor=rmsb.tensor, offset=rmsb[:].offset,
                    ap=rmsb[:].ap + [[0, DH]])
                nc.vector.tensor_mul(qkrot[:], qkrot[:], rview)

                # transpose pairs into qTs/kTs; two token tiles share one
                # psum tile so the (DVE) drain copies are [128, 256]
                for pr in range(4):
                    dst = (qTs if pr < 2 else kTs)[pr % 2]
                    for tp2 in range(2):
                        tp = psp.tile([128, 2, 128], BF16, tag="tp",
                                      name=f"tp{qtr}_{pr}_{tp2}")
                        for t4 in (2 * tp2, 2 * tp2 + 1):
                            nc.tensor.transpose(
                                tp[:, t4 % 2],
                                qkrot[:, t4, 2 * pr : 2 * pr + 2, :],
                                ident[:])
                        t = t0 + 2 * tp2
                        nc.vector.tensor_copy(
                            dst[:, t * 128 : (t + 2) * 128], tp[:])

            # ------- P3: one head-pair of one query quarter (512 cols) -------
            def emit_p3(r, pair, scps, atps, pbsb, recp, denbp):
                jmax = 4 * r + 4
                kt = kTs[pair]
                qt = qTs[pair]
                ats = [atps.tile([DH + 1, 512], F32, tag="at",
                                 name=f"at{r}_{pair}_{hi}") for hi in range(2)]
                for j in range(jmax):
                    wlo = max(j * 128 - 512 * r, 0)  # col start within quarter
                    sc = scps.tile([128, 2, 512], F32, tag="sc",
                                   name=f"sc{r}_{pair}_{j}")
                    for hi in range(2):
                        nc.tensor.matmul(
                            sc[:, hi, wlo:512],
                            kt[64 * hi : 64 * hi + 64,
                               j * 128 : (j + 1) * 128],
                            qt[64 * hi : 64 * hi + 64,
                               512 * r + wlo : 512 * r + 512],
                            start=True, stop=True)
                    pb = pbsb.tile([128, 2, 512], BF16, tag="pb",
                                   name=f"pb{r}_{pair}_{j}")
                    nc.scalar.activation(
                        pb[:, :, wlo:512], sc[:, :, wlo:512],
                        mybir.ActivationFunctionType.Exp, scale=1.0 / 8.0)
                    if r == 3 and j == 0:
                        # mask: last query frame can't see key frame 0
                        nc.vector.memset(pb[:, :, 384:512], 0.0)
                    for hi in range(2):
                        nc.tensor.matmul(
                            ats[hi][:, wlo:512],
                            vsb[:, j, 2 * pair + hi, :],
                            pb[:, hi, wlo:512],
                            start=(j == 0), stop=(j == jmax - 1))
                # denominator reciprocal, broadcast over the head's 64 rows,
                # fused into the PSUM->attnStack move
                for hi in range(2):
                    at = ats[hi]
                    rden = recp.tile([1, 512], F32, tag="rden",
                                     name=f"rden{r}_{pair}_{hi}")
                    nc.vector.reciprocal(rden[:], at[DH : DH + 1, :])
                    denb = denbp.tile([64, 512], F32, tag="denb",
                                      name=f"denb{r}_{pair}_{hi}")
                    nc.gpsimd.partition_broadcast(denb[:], rden[:])
                    nc.vector.tensor_mul(
                        attnStack[pair][64 * hi : 64 * hi + 64,
                                        512 * r : 512 * r + 512],
                        at[0:DH, :], denb[:])

            # -------- AllGather of one head-pair's quarter channels ---------
            def emit_ag(r, pair):
                ag_in = dram.tile([128, 512], BF16, name=f"agi{r}_{pair}")
                nc.sync.dma_start(
                    ag_in[:], attnStack[pair][:, 512 * r : 512 * r + 512])
                ago = dram.tile([4, 128, 512], BF16, name=f"ago{r}_{pair}")
                nc.gpsimd.collective_compute(
                    "AllGather", mybir.AluOpType.bypass,
                    replica_groups=GROUPS,
                    ins=[ag_in[:].opt()], outs=[ago[:].opt()])
                return ago

            def emit_p4(r, agos, agp, osb, scps):
                agsb = []
                for pair in range(2):
                    a = agp.tile([128, 4, 128], BF16, tag=f"agsb{pair}",
                                 name=f"agsb{r}_{pair}")
                    agov = agos[pair][:].rearrange("s p q -> p s q")
                    nc.sync.dma_start(a[:], agov[:, :, bass.ds(qv, 128)])
                    agsb.append(a)
                ost = osb.tile([128, D], F32, tag="ost", name=f"ost{r}")
                for o0 in range(0, D, 512):
                    op = scps.tile([128, 512], F32, tag="sc",
                                   name=f"op{r}_{o0}")
                    for pair in range(2):
                        for s in range(4):
                            nc.tensor.matmul(
                                op[:],
                                agsb[pair][:, s, :],
                                wos[:, s * 2 + pair, o0 : o0 + 512],
                                start=(pair == 0 and s == 0),
                                stop=(pair == 1 and s == 3))
                    nc.vector.tensor_copy(ost[:, o0 : o0 + 512], op[:])
                nc.sync.dma_start(
                    out[128 * r : 128 * (r + 1), :], ost[:])

            # ---------------- emission schedule ----------------
            # P1 matmuls run ahead of the (vector-bound) P2 of the previous
            # quarter so the tensor queue never waits on rope.
            with (
                tc.tile_pool(name="p12sb", bufs=2) as p12sb,
                tc.tile_pool(name="p12ps", bufs=2, space="PSUM") as p12ps,
                tc.tile_pool(name="tpps", bufs=2, space="PSUM") as tpps,
            ):
                raws = []
                raws.append(emit_p1_quarter(0, p12sb, p12ps))
                raws.append(emit_p1_quarter(1, p12sb, p12ps))
                emit_p2_quarter(0, *raws[0], p12sb, tpps)
                raws.append(emit_p1_quarter(2, p12sb, p12ps))
                emit_p2_quarter(1, *raws[1], p12sb, tpps)
                raws.append(emit_p1_quarter(3, p12sb, p12ps))
                emit_p2_quarter(2, *raws[2], p12sb, tpps)
                emit_p2_quarter(3, *raws[3], p12sb, tpps)

            nc.scalar.dma_start(
                wos[:], wo[:].rearrange("(kc p) o -> p kc o", p=128))

            with (
                tc.tile_pool(name="scps", bufs=3, space="PSUM") as scps,
                tc.tile_pool(name="atps", bufs=2, space="PSUM") as atps,
                tc.tile_pool(name="pbsb", bufs=4) as pbsb,
                tc.tile_pool(name="recp", bufs=2) as recp,
                tc.tile_pool(name="denbp", bufs=2) as denbp,
                tc.tile_pool(name="osb", bufs=2) as osb,
                tc.tile_pool(name="agp", bufs=2) as agp,
            ):
                agos = {}
                for r in range(4):
                    emit_p3(r, 0, scps, atps, pbsb, recp, denbp)
                    agos[(r, 0)] = emit_ag(r, 0)
                    emit_p3(r, 1, scps, atps, pbsb, recp, denbp)
                    agos[(r, 1)] = emit_ag(r, 1)
                    if r >= 1:
                        emit_p4(r - 1, [agos[(r - 1, 0)], agos[(r - 1, 1)]],
                                agp, osb, scps)
                emit_p4(3, [agos[(3, 0)], agos[(3, 1)]], agp, osb, scps)

    nc.compile()
    return nc


_PROG_CACHE = {}


def _get_program(key):
    if key not in _PROG_CACHE:
        _PROG_CACHE[key] = build_program(*key)
    return _PROG_CACHE[key]


def _host_inputs(x, W_qkv, b_qkv, W_out, b_out, q_gamma, k_gamma):
    import ml_dtypes
    mmnp = ml_dtypes.bfloat16
    x = np.asarray(x, dtype=np.float32)
    W_qkv = np.asarray(W_qkv, dtype=np.float32)
    b_qkv = np.asarray(b_qkv, dtype=np.float32)
    W_out = np.asarray(W_out, dtype=np.float32)
    q_gamma = np.asarray(q_gamma, dtype=np.float32)
    k_gamma = np.asarray(k_gamma, dtype=np.float32)

    apply_gamma = not (np.all(q_gamma == 1.0) and np.all(k_gamma == 1.0))
    qkv_bias = bool(np.any(b_qkv))

    # rope tables: pos = t*128 + p, replicated over the 8 q/k head slots
    pos = np.arange(L, dtype=np.float64).reshape(NT, 128).T  # [128, NT]
    inv = 1.0 / (ROPE_THETA ** (np.arange(32, dtype=np.float64) / 32.0))
    ang = pos[:, :, None] * inv[None, None, :]               # [128, NT, 32]
    cosb = np.broadcast_to(
        np.cos(ang)[:, :, None, :], (128, NT, 8, 32)).astype(mmnp).copy()
    sinb = np.broadcast_to(
        np.sin(ang)[:, :, None, :], (128, NT, 8, 32)).astype(mmnp).copy()

    Wq = W_qkv[0 * D : 1 * D]
    Wk = W_qkv[1 * D : 2 * D]
    Wv = W_qkv[2 * D : 3 * D]
    WoT = np.ascontiguousarray(W_out.T)  # [d_in, d_out]

    in_maps = []
    for c in range(N_CORES):
        b = c // 4
        h0 = 4 * (c % 4)
        rows = slice(h0 * DH, (h0 + HPC) * DH)
        wq_c = np.ascontiguousarray(
            np.concatenate([Wq[rows], Wk[rows], Wv[rows]], axis=0).T)
        m = {
            "xT": np.ascontiguousarray(x[b].T).astype(mmnp),
            "wq": wq_c.astype(mmnp),
            "wo": WoT.astype(mmnp),
            "cosb": cosb,
            "sinb": sinb,
            "qoff": np.array([[128 * (c % 4)]], dtype=np.uint32),
        }
        if apply_gamma:
            m["gam"] = np.ascontiguousarray(
                np.concatenate([np.broadcast_to(q_gamma, (4, DH)),
                                np.broadcast_to(k_gamma, (4, DH))], axis=0))
        if qkv_bias:
            m["bqk"] = np.ascontiguousarray(np.concatenate(
                [b_qkv[0 * D : 1 * D][rows], b_qkv[1 * D : 2 * D][rows],
                 b_qkv[2 * D : 3 * D][rows]]))
        in_maps.append(m)

    key = (apply_gamma, qkv_bias)
    return key, in_maps


def _assemble(results, b_out):
    y = np.empty((B, L, D), dtype=np.float32)
    for c in range(N_CORES):
        b = c // 4
        rank = c % 4
        o = results[c]["out"]
        for r in range(4):
            rows = slice(512 * r + 128 * rank, 512 * r + 128 * rank + 128)
            y[b, rows, :] = o[128 * r : 128 * r + 128]
    b_out = np.asarray(b_out, dtype=np.float32)
    if np.any(b_out):
        y += b_out
    return y


def _install_ntff_hook():
    """Register the axon NTFF profiling hook (the container's antenv stub
    lacks axon_hooks; replicate what trn_boot would have registered)."""
    import sys
    import types
    try:
        from antenv.axon_hooks import get_axon_ntff_profile_hook  # noqa: F401
        return
    except ImportError:
        pass
    try:
        from trn_agent_boot.trn_boot import _ntff_profile_via_ctypes
        hook = _ntff_profile_via_ctypes("/opt/axon/libaxon_pjrt.so")
    except Exception:
        hook = None
    import antenv
    mod = types.ModuleType("antenv.axon_hooks")
    mod.get_axon_ntff_profile_hook = lambda: hook
    mod.set_axon_ntff_profile_hook = lambda h: None
    antenv.axon_hooks = mod
    sys.modules["antenv.axon_hooks"] = mod


def kernel(x, W_qkv, b_qkv, W_out, b_out, q_gamma, k_gamma, _trace=False):
    from concourse.bass_utils import run_bass_kernel_spmd
    if _trace:
        _install_ntff_hook()

    key, in_maps = _host_inputs(x, W_qkv, b_qkv, W_out, b_out,
                                q_gamma, k_gamma)
    nc = _get_program(key)
    res = run_bass_kernel_spmd(nc, in_maps, core_ids=list(range(N_CORES)),
                               trace=_trace,
                               trace_cores=list(range(N_CORES)) if _trace else None)
    y = _assemble(res.results, b_out)
    if _trace:
        return y, res
    return y
